# revision 9
# baseline (speedup 1.0000x reference)
"""Trainium2 Bass kernel for nn_Attention_b (tanh-attention with masked_scatter).

Data-parallel over batch: each of 8 NeuronCores owns 4 batches. Heavy
operands travel in fp16 (end-to-end rel err ~1e-3): halves HBM traffic and
doubles DVE throughput, and the whole per-core h_i slice fits in SBUF.

Per core:
  front    warm-up AllGather (brings up CC rings / absorbs core skew while
           compute runs), all h_i + sel DMAs issued up front
  phase 1  (all chunks, back-to-back) z = W1 @ h_i (+cb bias via act);
           m = tanh(z); y = u . m; AllGather y chunk (fp16); the gathered
           result lands via the idle sync DMA queue
  phase 2  (after all phase 1) masked_scatter selection (one-hot matmul
           against gathered scores) + chunk-local softmax stats
  phase 3  sacc_i += e * h_i  (DVE fused multiply-reduce, fp16)
  final    flash-style combine of chunk partials, transpose, store
"""
import sys

for _p in ("/opt/trn_rl_repo",):
    if _p not in sys.path:
        sys.path.insert(0, _p)

import numpy as np

import concourse.bacc as bacc
import concourse.tile as tile
from concourse import mybir
from concourse.bass_utils import run_bass_kernel_spmd
from concourse.dve_ops import TENSOR_TENSOR_REDUCE
from concourse.masks import make_identity

NCORES = 8
B, S, H, A = 32, 2048, 1024, 256
BL = B // NCORES          # local batches per core
NEG = np.float32(-60000.0)   # fits fp16; exp(NEG-max) == 0 regardless

f32 = mybir.dt.float32
f16 = mybir.dt.float16


def build_kernel(S=S, H=H, A=A, clist=None, sel_gpsimd=True):
    KT = H // 128             # contraction tiles
    AT = A // 128             # score tiles
    if clist is None:
        clist = [128, 128, 256] + [512] * 3
    offs = np.concatenate([[0], np.cumsum(clist)]).tolist()
    NCH = len(clist)
    assert offs[-1] == S and H % 128 == 0 and A % 128 == 0

    nc = bacc.Bacc("TRN2", target_bir_lowering=False, debug=False,
                   num_devices=NCORES)

    hi5 = nc.declare_dram_parameter("hi5", [128, KT * BL * S], f16,
                                    isOutput=False)
    w1t = nc.declare_dram_parameter("w1t", [H, A], f16, isOutput=False)
    cb2 = nc.declare_dram_parameter("cb2", [128, AT, BL], f32, isOutput=False)
    u2 = nc.declare_dram_parameter("u2", [128, AT], f16, isOutput=False)
    sel = nc.declare_dram_parameter("sel", [B + 1, BL, S], f16,
                                    isOutput=False)
    out = nc.declare_dram_parameter("out", [BL, H], f32, isOutput=True)

    with tile.TileContext(nc) as tc:
        with (
            tc.tile_pool(name="consts", bufs=1) as cp,
            tc.tile_pool(name="m", bufs=2) as mp,
            tc.tile_pool(name="tiny", bufs=2) as tp,
            tc.tile_pool(name="sely", bufs=2) as syp,
            tc.tile_pool(name="ebc", bufs=2) as ebp,
            tc.tile_pool(name="sacc", bufs=2) as sap,
            tc.tile_pool(name="pz", bufs=2, space="PSUM") as pz,
            tc.tile_pool(name="py", bufs=2, space="PSUM") as py,
            tc.tile_pool(name="dram", bufs=NCH, space="DRAM") as dp,
        ):
            # ---- preload replicated constants
            w1_sb = cp.tile([128, KT, A], f16)
            nc.sync.dma_start(
                out=w1_sb, in_=w1t.rearrange("(t p) a -> p t a", p=128))
            u_sb = cp.tile([128, AT], f16)
            nc.sync.dma_start(out=u_sb, in_=u2[:, :])
            cb_sb = cp.tile([128, AT, BL], f32)
            nc.sync.dma_start(out=cb_sb, in_=cb2[:, :, :])
            ident = cp.tile([128, 128], f32)
            make_identity(nc, ident)
            ones_sb = cp.tile([B + 1, 1], f16)
            nc.vector.memset(ones_sb, 1.0)
            wup = cp.tile([128, 512], f16)
            nc.vector.memset(wup, 0.0)

            # ---- warm-up: align cores / bring up CC rings early
            wg_in = dp.tile([1], f16, tag="wgin")
            nc.scalar.dma_start(out=wg_in.rearrange("(o n) -> o n", o=1),
                                in_=ones_sb[0:1, 0:1])
            wg_out = dp.tile([8], f16, tag="wgout", addr_space="Shared")
            nc.gpsimd.collective_compute(
                "AllGather", mybir.AluOpType.bypass,
                ins=[wg_in[:]], outs=[wg_out[:]],
                replica_groups=[list(range(NCORES))],
            )

            # ---- all input streams issued up front (nothing gates them)
            hi_all = cp.tile([128, KT, BL, S], f16)
            CB = 4096
            for o in range(0, KT * BL * S, CB * 2):
                for q, eng in ((0, nc.sync), (1, nc.scalar)):
                    lo = o + q * CB
                    if lo < KT * BL * S:
                        eng.dma_start(
                            out=hi_all.rearrange("p t b s -> p (t b s)")
                                      [:, lo : lo + CB],
                            in_=hi5[:, lo : lo + CB])
            sel_all = cp.tile([B + 1, BL, S], f16)
            nc.scalar.dma_start(out=sel_all, in_=sel[:, :, :])

            # PE p-state warm-up while DMAs stream
            wup_ps = pz.tile([128, 2, 512], f32, tag="z")
            for i in range(10):
                nc.tensor.matmul(
                    wup_ps[:, i % 2, :], wup[:, 0:128], wup[:, 0:512],
                    start=True, stop=True)

            # ---- per-chunk softmax stats (combined once at the end)
            mall = cp.tile([1, BL, NCH], f32)
            lall = cp.tile([1, BL, NCH], f32)
            saccs = [cp.tile([128, KT, BL], f32, name=f"sacc{i}")
                     for i in range(NCH)]

            def hi_c(i):
                Ci, off = clist[i], offs[i]
                return hi_all[:, :, :, off : off + Ci]

            def phase1(i):
                Ci, off = clist[i], offs[i]
                hi_sb = hi_c(i)
                m_r = mp.tile([128, AT, BL, Ci], f16, tag="m")
                for at in range(AT):
                    for bp in range(BL // 2):
                        z_ps = pz.tile([128, 2, Ci], f32, tag="z")
                        for b2 in range(2):
                            b = bp * 2 + b2
                            for kt in range(KT):
                                nc.tensor.matmul(
                                    z_ps[:, b2, :],
                                    w1_sb[:, kt, at * 128 : (at + 1) * 128],
                                    hi_sb[:, kt, b, :],
                                    start=(kt == 0), stop=(kt == KT - 1),
                                )
                        for b2 in range(2):
                            b = bp * 2 + b2
                            nc.scalar.activation(
                                out=m_r[:, at, b, :], in_=z_ps[:, b2, :],
                                func=mybir.ActivationFunctionType.Tanh,
                                bias=cb_sb[:, at, b : b + 1], scale=1.0,
                            )
                y_sb = tp.tile([1, BL, Ci], f16, tag="ysb")
                for bp in range(BL // 2):
                    y_ps = py.tile([1, 2, Ci], f32, tag="y")
                    for b2 in range(2):
                        b = bp * 2 + b2
                        for at in range(AT):
                            nc.tensor.matmul(
                                y_ps[:, b2 : b2 + 1, :],
                                u_sb[:, at : at + 1],
                                m_r[:, at, b : b + 1, :],
                                start=(at == 0), stop=(at == AT - 1),
                            )
                    nc.scalar.activation(
                        out=y_sb[:, 2 * bp : 2 * bp + 2, :], in_=y_ps,
                        func=mybir.ActivationFunctionType.Copy)

                ag_in = dp.tile([BL * Ci], f16, tag="agin")
                nc.scalar.dma_start(
                    out=ag_in.rearrange("(o n) -> o n", o=1),
                    in_=y_sb.rearrange("p b s -> p (b s)"))
                ag_out = dp.tile([B * Ci], f16, tag="agout",
                                 addr_space="Shared")
                nc.gpsimd.collective_compute(
                    "AllGather", mybir.AluOpType.bypass,
                    ins=[ag_in[:]], outs=[ag_out[:]],
                    replica_groups=[list(range(NCORES))],
                )
                # land the gathered scores via the now-idle sync queue
                y32 = cp.tile([B + 1, Ci], f16, name=f"y32_{i}")
                nc.gpsimd.memset(y32[B : B + 1, :], 1.0)
                nc.sync.dma_start(
                    out=y32[:B, :],
                    in_=ag_out.rearrange("(j s) -> j s", s=Ci))
                return dict(y32=y32, i=i, Ci=Ci)

            def phase2(c):
                i, Ci = c["i"], c["Ci"]
                off = offs[i]
                sel_c = sel_all[:, :, off : off + Ci]
                y32 = c["y32"]
                selY = syp.tile([B + 1, BL, Ci], f16, tag="selY")
                eng = nc.gpsimd if sel_gpsimd else nc.vector
                eng.tensor_mul(
                    selY, sel_c,
                    y32.rearrange("j (o s) -> j o s", o=1)
                       .broadcast_to([B + 1, BL, Ci]))
                cmax = tp.tile([1, BL], f32, tag="cmax")
                bt_hs = []
                for hf in range(2):
                    bt_ps = py.tile([1, 2, Ci], f32, tag="y")
                    for b2 in range(2):
                        nc.tensor.matmul(
                            bt_ps[:, b2 : b2 + 1, :], ones_sb,
                            selY[:, 2 * hf + b2 : 2 * hf + b2 + 1, :],
                            start=True, stop=True)
                    nc.vector.tensor_reduce(
                        out=cmax.rearrange("p (b o) -> p b o", o=1)
                                [:, 2 * hf : 2 * hf + 2],
                        in_=bt_ps,
                        axis=mybir.AxisListType.X, op=mybir.AluOpType.max)
                    bt_hs.append(bt_ps)
                nc.vector.tensor_copy(mall[:, :, i], cmax)
                nmnew = tp.tile([1, BL], f32, tag="nmnew")
                nc.vector.tensor_scalar_mul(nmnew, cmax, -1.0)
                e4 = tp.tile([1, BL, Ci], f16, tag="e4")
                for b in range(BL):
                    nc.scalar.activation(
                        out=e4[:, b, :], in_=bt_hs[b // 2][:, b % 2, :],
                        func=mybir.ActivationFunctionType.Exp,
                        bias=nmnew[:, b : b + 1], scale=1.0,
                        accum_out=lall[:, b, i : i + 1])
                e_bc = ebp.tile([128, BL, Ci], f16, tag="ebc")
                nc.gpsimd.partition_broadcast(
                    e_bc.rearrange("p b s -> p (b s)"),
                    e4.rearrange("p b s -> p (b s)"))
                c["ebc"] = e_bc

            def phase3(c):
                i, Ci = c["i"], c["Ci"]
                sacc_i = saccs[i]
                ttr_scr = tp.tile([128, 1], f16, tag="ttrscr")
                hi_sb = hi_c(i)
                e_bc_all = c["ebc"]
                for b in range(BL):
                    e_bc = e_bc_all[:, b, :]
                    for kt in range(KT):
                        nc.vector._custom_dve(
                            TENSOR_TENSOR_REDUCE,
                            out=ttr_scr.broadcast_to([128, Ci]),
                            in0=hi_sb[:, kt, b, :],
                            in1=e_bc,
                            s0=0.0, s1=1.0,
                            accum_out=sacc_i[:, kt, b : b + 1],
                        )

            carries = [phase1(i) for i in range(NCH)]
            for c in carries:
                phase2(c)
                phase3(c)

            # ---- finalize: combine chunk partials, divide, transpose, store
            M = tp.tile([1, BL], f32, tag="cmax")
            nc.vector.tensor_reduce(
                out=M.rearrange("p (b o) -> p b o", o=1), in_=mall,
                axis=mybir.AxisListType.X, op=mybir.AluOpType.max)
            nM = tp.tile([1, BL], f32, tag="nmnew")
            nc.vector.tensor_scalar_mul(nM, M, -1.0)
            w = tp.tile([1, BL, NCH], f32, tag="w")
            for b in range(BL):
                nc.scalar.activation(
                    out=w[:, b, :], in_=mall[:, b, :],
                    func=mybir.ActivationFunctionType.Exp,
                    bias=nM[:, b : b + 1], scale=1.0)
            wl = tp.tile([1, BL, NCH], f32, tag="wl")
            nc.vector.tensor_mul(wl, w, lall)
            lsum = tp.tile([1, BL], f32, tag="lsum")
            nc.vector.tensor_reduce(
                out=lsum.rearrange("p (b o) -> p b o", o=1), in_=wl,
                axis=mybir.AxisListType.X, op=mybir.AluOpType.add)
            il = tp.tile([1, BL], f32, tag="il")
            nc.vector.reciprocal(il, lsum)
            wn = tp.tile([1, BL, NCH], f32, tag="wn")
            for b in range(BL):
                nc.vector.tensor_scalar_mul(wn[:, b, :], w[:, b, :],
                                            il[:, b : b + 1])
            wbc = ebp.tile([128, BL, NCH], f32, tag="wbcf")
            nc.gpsimd.partition_broadcast(
                wbc.rearrange("p b n -> p (b n)"),
                wn.rearrange("p b n -> p (b n)"))
            sfin = sap.tile([128, KT, BL], f32, tag="sacc")
            for i in range(NCH):
                for b in range(BL):
                    if i == 0:
                        nc.vector.tensor_scalar_mul(
                            sfin[:, :, b], saccs[0][:, :, b],
                            wbc[:, b, 0:1])
                    else:
                        tmp = tp.tile([128, KT], f32, tag="ftmp")
                        nc.vector.tensor_scalar_mul(
                            tmp, saccs[i][:, :, b], wbc[:, b, i : i + 1])
                        nc.vector.tensor_add(
                            sfin[:, :, b], sfin[:, :, b], tmp)
            t_ps = py.tile([KT * BL, 128], f32, tag="y")
            nc.tensor.transpose(
                t_ps, sfin.rearrange("p t b -> p (t b)"), ident)
            t_sb = tp.tile([KT * BL, 128], f32, tag="tsb")
            nc.vector.tensor_copy(t_sb, t_ps)
            for t in range(KT):
                nc.sync.dma_start(
                    out=out[:, t * 128 : (t + 1) * 128],
                    in_=t_sb[t * BL : (t + 1) * BL, :])

    nc.compile()
    _split_pe_waits(nc)
    return nc


def _split_pe_waits(nc):
    """TRN2 PE instructions (S3_LW encoding) take a single sync-wait slot.
    Bacc's legalization misses some Matmults; hoist excess waits onto
    dedicated PE NoOps inserted directly before the offender."""
    for f in nc.m.functions:
        for bb in f.blocks:
            insts = bb.instructions
            i = 0
            while i < len(insts):
                ins = insts[i]
                if type(ins).__name__ in ("InstMatmult", "InstNoOp") and \
                        ins.engine == mybir.EngineType.PE:
                    si = ins.sync_info
                    if si is not None and len(si.on_wait) > 1:
                        extra, keep = si.on_wait[:-1], si.on_wait[-1:]
                        for w in extra:
                            nop = mybir.InstNoOp(
                                name=nc.get_next_instruction_name(),
                                ins=[], outs=[])
                            nop.engine = ins.engine
                            nop.sync_info = mybir.SyncInfo(
                                on_wait=[w], on_update=[])
                            nc.register_instruction(nop)
                            insts.insert(i, nop)
                            i += 1
                        si.on_wait = keep
                i += 1


def prep_inputs(h_i, h_t, mask, W, b, u, S=S, H=H, A=A, clist=None):
    """Shard + lay out the full inputs for the 8 cores (heavy data fp16)."""
    h_i = np.asarray(h_i, np.float32)
    h_t = np.asarray(h_t, np.float32)
    mask = np.asarray(mask, bool)
    W = np.asarray(W, np.float32)
    b = np.asarray(b, np.float32)
    u = np.asarray(u, np.float32)

    KT = H // 128
    AT = A // 128
    w1t = np.ascontiguousarray(W[:, :H].T).astype(np.float16)   # [H, A]
    cb = h_t @ W[:, H:].T + b                                   # [B, A]
    cb2s = np.ascontiguousarray(
        cb.reshape(B, AT, 128).transpose(2, 1, 0))              # [128, AT, B]
    u2 = np.ascontiguousarray(
        u[:, 0].reshape(AT, 128).T).astype(np.float16)          # [128, AT]

    pos = np.clip(np.cumsum(mask.astype(np.int64), axis=0) - 1, 0, None)
    onehot = (np.arange(B)[None, :, None] == pos[:, None, :]) & mask[:, None, :]
    selall = onehot.astype(np.float16)                          # [B, B, S]
    negall = np.where(mask, np.float16(0), np.float16(NEG))     # [B, S]
    sel33 = np.concatenate([selall, negall[:, None, :]], axis=1)  # [B, B+1, S]

    h16 = h_i.astype(np.float16)
    in_maps = []
    for c in range(NCORES):
        bs = slice(c * BL, (c + 1) * BL)
        # hi5[p, (t, b, s)] = h_i[b, s, t*128+p]  (single full-S block)
        hcf = h16[bs].reshape(BL, S, KT, 128)
        hi5 = np.ascontiguousarray(
            hcf.transpose(3, 2, 0, 1).reshape(128, KT * BL * S))
        in_maps.append({
            "hi5": hi5,
            "w1t": w1t,
            "cb2": np.ascontiguousarray(cb2s[:, :, bs]),
            "u2": u2,
            "sel": np.ascontiguousarray(sel33[bs].transpose(1, 0, 2)),
        })
    return in_maps


_NC_CACHE = {}


CLIST = [128, 128, 256] + [512] * 3


def _get_nc():
    if "nc" not in _NC_CACHE:
        _NC_CACHE["nc"] = build_kernel(clist=CLIST)
    return _NC_CACHE["nc"]


def kernel(h_i, h_t, mask, W, b, u):
    nc = _get_nc()
    in_maps = prep_inputs(h_i, h_t, mask, W, b, u, clist=CLIST)
    res = run_bass_kernel_spmd(nc, in_maps, list(range(NCORES)))
    return np.concatenate([res.results[c]["out"] for c in range(NCORES)],
                          axis=0)


# revision 13
# speedup vs baseline: 1.3110x; 1.3110x over previous
"""Trainium2 Bass kernel for nn_Attention_b (tanh-attention with masked_scatter).

Data-parallel over batch: each of 8 NeuronCores owns 4 batches. Heavy
operands travel in fp16 (end-to-end rel err ~2e-3): halves HBM traffic,
doubles DVE throughput, and the whole per-core h_i slice fits in SBUF.

Per core (S split into four 512-column chunks):
  front    warm-up AllGather (brings up CC rings / absorbs core skew while
           compute runs); all h_i chunk DMAs + sel DMA issued up front
  phase 1  x4, back-to-back: z = W1 @ h_i (+cb bias via act); m = tanh(z);
           y = u . m; y lands in a DRAM score buffer per AllGather group
  comm     3 grouped AllGathers (chunk 0 | chunks 1-2 | chunk 3), each
           trigger interleaved with phase-2/3 gpsimd work so no engine
           queue ever parks behind an unfinished collective
  phase 2  masked_scatter selection (one-hot matmul against gathered
           scores) + chunk-local softmax stats
  phase 3  sacc_i += e * h_i  (DVE fused multiply-reduce, fp16)
  final    flash-style combine of chunk partials, transpose, store
"""
import sys

for _p in ("/opt/trn_rl_repo",):
    if _p not in sys.path:
        sys.path.insert(0, _p)

import numpy as np

import concourse.bacc as bacc
import concourse.tile as tile
from concourse import mybir
from concourse.bass_utils import run_bass_kernel_spmd
from concourse.dve_ops import TENSOR_TENSOR_REDUCE
from concourse.masks import make_identity

NCORES = 8
B, S, H, A = 32, 2048, 1024, 256
BL = B // NCORES          # local batches per core
NEG = np.float32(-60000.0)   # fits fp16; exp(NEG-max) == 0 regardless

f32 = mybir.dt.float32
f16 = mybir.dt.float16

CI = 512
NCH = S // CI             # 4 chunks
AGROUPS = [[0], [1, 2], [3]]   # AllGather groups, in chunk indices


def build_kernel(S=S, H=H, A=A, sel_gpsimd=True):
    KT = H // 128             # contraction tiles
    AT = A // 128             # score tiles

    nc = bacc.Bacc("TRN2", target_bir_lowering=False, debug=False,
                   num_devices=NCORES)

    hi5 = nc.declare_dram_parameter("hi5", [128, KT * BL * S], f16,
                                    isOutput=False)
    w1t = nc.declare_dram_parameter("w1t", [H, A], f16, isOutput=False)
    cb2 = nc.declare_dram_parameter("cb2", [128, AT, BL], f32, isOutput=False)
    u2 = nc.declare_dram_parameter("u2", [128, AT], f16, isOutput=False)
    sel = nc.declare_dram_parameter("sel", [B + 1, BL, S], f16,
                                    isOutput=False)
    out = nc.declare_dram_parameter("out", [BL, H], f32, isOutput=True)

    with tile.TileContext(nc) as tc:
        with (
            tc.tile_pool(name="consts", bufs=1) as cp,
            tc.tile_pool(name="m", bufs=2) as mp,
            tc.tile_pool(name="tiny", bufs=2) as tp,
            tc.tile_pool(name="sely", bufs=2) as syp,
            tc.tile_pool(name="ebc", bufs=2) as ebp,
            tc.tile_pool(name="sacc", bufs=2) as sap,
            tc.tile_pool(name="pz", bufs=2, space="PSUM") as pz,
            tc.tile_pool(name="py", bufs=2, space="PSUM") as py,
            tc.tile_pool(name="dram", bufs=max(4, len(AGROUPS)),
                         space="DRAM") as dp,
        ):
            # ---- preload replicated constants
            w1_sb = cp.tile([128, KT, A], f16)
            nc.sync.dma_start(
                out=w1_sb, in_=w1t.rearrange("(t p) a -> p t a", p=128))
            u_sb = cp.tile([128, AT], f16)
            nc.sync.dma_start(out=u_sb, in_=u2[:, :])
            cb_sb = cp.tile([128, AT, BL], f32)
            nc.sync.dma_start(out=cb_sb, in_=cb2[:, :, :])
            ident = cp.tile([128, 128], f32)
            make_identity(nc, ident)
            ones_sb = cp.tile([B + 1, 1], f16)
            nc.vector.memset(ones_sb, 1.0)
            wup = cp.tile([128, 512], f16)
            nc.vector.memset(wup, 0.0)

            # ---- warm-up: align cores / bring up CC rings early
            wg_in = dp.tile([1], f16, tag="wgin")
            nc.scalar.dma_start(out=wg_in.rearrange("(o n) -> o n", o=1),
                                in_=ones_sb[0:1, 0:1])
            wg_out = dp.tile([8], f16, tag="wgout", addr_space="Shared")
            nc.gpsimd.collective_compute(
                "AllGather", mybir.AluOpType.bypass,
                ins=[wg_in[:]], outs=[wg_out[:]],
                replica_groups=[list(range(NCORES))],
            )

            # ---- all input streams issued up front, h_i chunk-major
            hi_all = cp.tile([128, NCH, KT, BL, CI], f16)
            for i in range(NCH):
                nc.sync.dma_start(
                    out=hi_all[:, i].rearrange("p t b s -> p (t b s)"),
                    in_=hi5[:, KT * BL * CI * i : KT * BL * CI * (i + 1)])
            sel_all = cp.tile([B + 1, BL, S], f16)
            nc.scalar.dma_start(out=sel_all, in_=sel[:, :, :])

            # PE p-state warm-up while DMAs stream
            wup_ps = pz.tile([128, 2, CI], f32, tag="z")
            for i in range(10):
                nc.tensor.matmul(
                    wup_ps[:, i % 2, :], wup[:, 0:128], wup[:, 0:512],
                    start=True, stop=True)

            # ---- per-chunk softmax stats (combined once at the end)
            mall = cp.tile([1, BL, NCH], f32)
            lall = cp.tile([1, BL, NCH], f32)
            saccs = [cp.tile([128, KT, BL], f32, name=f"sacc{i}")
                     for i in range(NCH)]
            y32s = [cp.tile([B + 1, CI], f16, name=f"y32_{i}")
                    for i in range(NCH)]
            for i in range(NCH):
                nc.gpsimd.memset(y32s[i][B : B + 1, :], 1.0)

            # per-AllGather-group DRAM score buffers
            ag_ins, ag_outs, gwidth, goff = [], [], [], []
            for g, chunks in enumerate(AGROUPS):
                wdt = len(chunks) * CI
                gwidth.append(wdt)
                goff.append(chunks[0] * CI)
                agi = dp.tile([BL * wdt], f16, tag=f"agin{g}",
                              name=f"agin{g}")
                ago = dp.tile([B * wdt], f16, tag=f"agout{g}",
                              name=f"agout{g}", addr_space="Shared")
                ag_ins.append(agi)
                ag_outs.append(ago)

            def phase1(i):
                hi_sb = hi_all[:, i]
                g = next(gi for gi, ch in enumerate(AGROUPS) if i in ch)
                m_r = mp.tile([128, AT, BL, CI], f16, tag="m")
                for at in range(AT):
                    for bp in range(BL // 2):
                        z_ps = pz.tile([128, 2, CI], f32, tag="z")
                        for b2 in range(2):
                            b = bp * 2 + b2
                            for kt in range(KT):
                                nc.tensor.matmul(
                                    z_ps[:, b2, :],
                                    w1_sb[:, kt, at * 128 : (at + 1) * 128],
                                    hi_sb[:, kt, b, :],
                                    start=(kt == 0), stop=(kt == KT - 1),
                                )
                        for b2 in range(2):
                            b = bp * 2 + b2
                            nc.scalar.activation(
                                out=m_r[:, at, b, :], in_=z_ps[:, b2, :],
                                func=mybir.ActivationFunctionType.Tanh,
                                bias=cb_sb[:, at, b : b + 1], scale=1.0,
                            )
                y_sb = tp.tile([1, BL, CI], f16, tag="ysb")
                for bp in range(BL // 2):
                    y_ps = py.tile([1, 2, CI], f32, tag="y")
                    for b2 in range(2):
                        b = bp * 2 + b2
                        for at in range(AT):
                            nc.tensor.matmul(
                                y_ps[:, b2 : b2 + 1, :],
                                u_sb[:, at : at + 1],
                                m_r[:, at, b : b + 1, :],
                                start=(at == 0), stop=(at == AT - 1),
                            )
                    nc.scalar.activation(
                        out=y_sb[:, 2 * bp : 2 * bp + 2, :], in_=y_ps,
                        func=mybir.ActivationFunctionType.Copy)
                # stage this chunk's scores into its group's DRAM buffer
                loc = i * CI - goff[g]
                nc.scalar.dma_start(
                    out=ag_ins[g].rearrange("(o b s) -> o b s", o=1,
                                            s=gwidth[g])
                                  [:, :, loc : loc + CI],
                    in_=y_sb)

            def ag_fire(g):
                nc.gpsimd.collective_compute(
                    "AllGather", mybir.AluOpType.bypass,
                    ins=[ag_ins[g][:]], outs=[ag_outs[g][:]],
                    replica_groups=[list(range(NCORES))],
                )
                # land gathered scores via the idle sync queue
                for i in AGROUPS[g]:
                    loc = i * CI - goff[g]
                    nc.sync.dma_start(
                        out=y32s[i][:B, :],
                        in_=ag_outs[g].rearrange("(j s) -> j s",
                                                 s=gwidth[g])
                                      [:, loc : loc + CI])

            def phase2(i):
                off = i * CI
                sel_c = sel_all[:, :, off : off + CI]
                y32 = y32s[i]
                selY = syp.tile([B + 1, BL, CI], f16, tag="selY")
                eng = nc.gpsimd if sel_gpsimd else nc.vector
                eng.tensor_mul(
                    selY, sel_c,
                    y32.rearrange("j (o s) -> j o s", o=1)
                       .broadcast_to([B + 1, BL, CI]))
                cmax = tp.tile([1, BL], f32, tag="cmax")
                bt_hs = []
                for hf in range(2):
                    bt_ps = py.tile([1, 2, CI], f32, tag="y")
                    for b2 in range(2):
                        nc.tensor.matmul(
                            bt_ps[:, b2 : b2 + 1, :], ones_sb,
                            selY[:, 2 * hf + b2 : 2 * hf + b2 + 1, :],
                            start=True, stop=True)
                    nc.vector.tensor_reduce(
                        out=cmax.rearrange("p (b o) -> p b o", o=1)
                                [:, 2 * hf : 2 * hf + 2],
                        in_=bt_ps,
                        axis=mybir.AxisListType.X, op=mybir.AluOpType.max)
                    bt_hs.append(bt_ps)
                nc.vector.tensor_copy(mall[:, :, i], cmax)
                nmnew = tp.tile([1, BL], f32, tag="nmnew")
                nc.vector.tensor_scalar_mul(nmnew, cmax, -1.0)
                e4 = tp.tile([1, BL, CI], f16, tag="e4")
                for b in range(BL):
                    nc.scalar.activation(
                        out=e4[:, b, :], in_=bt_hs[b // 2][:, b % 2, :],
                        func=mybir.ActivationFunctionType.Exp,
                        bias=nmnew[:, b : b + 1], scale=1.0,
                        accum_out=lall[:, b, i : i + 1])
                e_bc = ebp.tile([128, BL, CI], f16, tag="ebc")
                nc.gpsimd.partition_broadcast(
                    e_bc.rearrange("p b s -> p (b s)"),
                    e4.rearrange("p b s -> p (b s)"))
                return e_bc

            def phase3(i, e_bc_all):
                sacc_i = saccs[i]
                ttr_scr = tp.tile([128, 1], f16, tag="ttrscr")
                hi_sb = hi_all[:, i]
                for b in range(BL):
                    e_bc = e_bc_all[:, b, :]
                    for kt in range(KT):
                        nc.vector._custom_dve(
                            TENSOR_TENSOR_REDUCE,
                            out=ttr_scr.broadcast_to([128, CI]),
                            in0=hi_sb[:, kt, b, :],
                            in1=e_bc,
                            s0=0.0, s1=1.0,
                            accum_out=sacc_i[:, kt, b : b + 1],
                        )

            for i in range(NCH):
                phase1(i)
            # interleave AllGather triggers with phase-2/3 gpsimd work:
            # AG(g) fires, its chunks' phase2/3 run, next AG fires, ...
            ag_fire(0)
            for g in range(len(AGROUPS)):
                for i in AGROUPS[g]:
                    e_bc = phase2(i)
                    phase3(i, e_bc)
                # fire the next group's AG as soon as this group's gpsimd
                # work is issued (ring is free again by then)
                if g + 1 < len(AGROUPS):
                    ag_fire(g + 1)

            # ---- finalize: combine chunk partials, divide, transpose, store
            M = tp.tile([1, BL], f32, tag="cmax")
            nc.vector.tensor_reduce(
                out=M.rearrange("p (b o) -> p b o", o=1), in_=mall,
                axis=mybir.AxisListType.X, op=mybir.AluOpType.max)
            nM = tp.tile([1, BL], f32, tag="nmnew")
            nc.vector.tensor_scalar_mul(nM, M, -1.0)
            w = tp.tile([1, BL, NCH], f32, tag="w")
            for b in range(BL):
                nc.scalar.activation(
                    out=w[:, b, :], in_=mall[:, b, :],
                    func=mybir.ActivationFunctionType.Exp,
                    bias=nM[:, b : b + 1], scale=1.0)
            wl = tp.tile([1, BL, NCH], f32, tag="wl")
            nc.vector.tensor_mul(wl, w, lall)
            lsum = tp.tile([1, BL], f32, tag="lsum")
            nc.vector.tensor_reduce(
                out=lsum.rearrange("p (b o) -> p b o", o=1), in_=wl,
                axis=mybir.AxisListType.X, op=mybir.AluOpType.add)
            il = tp.tile([1, BL], f32, tag="il")
            nc.vector.reciprocal(il, lsum)
            wn = tp.tile([1, BL, NCH], f32, tag="wn")
            for b in range(BL):
                nc.vector.tensor_scalar_mul(wn[:, b, :], w[:, b, :],
                                            il[:, b : b + 1])
            wbc = ebp.tile([128, BL, NCH], f32, tag="wbcf")
            nc.gpsimd.partition_broadcast(
                wbc.rearrange("p b n -> p (b n)"),
                wn.rearrange("p b n -> p (b n)"))
            sfin = sap.tile([128, KT, BL], f32, tag="sacc")
            for i in range(NCH):
                for b in range(BL):
                    if i == 0:
                        nc.vector.tensor_scalar_mul(
                            sfin[:, :, b], saccs[0][:, :, b],
                            wbc[:, b, 0:1])
                    else:
                        tmp = tp.tile([128, KT], f32, tag="ftmp")
                        nc.vector.tensor_scalar_mul(
                            tmp, saccs[i][:, :, b], wbc[:, b, i : i + 1])
                        nc.vector.tensor_add(
                            sfin[:, :, b], sfin[:, :, b], tmp)
            t_ps = py.tile([KT * BL, 128], f32, tag="y")
            nc.tensor.transpose(
                t_ps, sfin.rearrange("p t b -> p (t b)"), ident)
            t_sb = tp.tile([KT * BL, 128], f32, tag="tsb")
            nc.vector.tensor_copy(t_sb, t_ps)
            for t in range(KT):
                nc.sync.dma_start(
                    out=out[:, t * 128 : (t + 1) * 128],
                    in_=t_sb[t * BL : (t + 1) * BL, :])

    nc.compile()
    _split_pe_waits(nc)
    return nc


def _split_pe_waits(nc):
    """TRN2 PE instructions (S3_LW encoding) take a single sync-wait slot.
    Bacc's legalization misses some Matmults; hoist excess waits onto
    dedicated PE NoOps inserted directly before the offender."""
    for f in nc.m.functions:
        for bb in f.blocks:
            insts = bb.instructions
            i = 0
            while i < len(insts):
                ins = insts[i]
                if type(ins).__name__ in ("InstMatmult", "InstNoOp") and \
                        ins.engine == mybir.EngineType.PE:
                    si = ins.sync_info
                    if si is not None and len(si.on_wait) > 1:
                        extra, keep = si.on_wait[:-1], si.on_wait[-1:]
                        for w in extra:
                            nop = mybir.InstNoOp(
                                name=nc.get_next_instruction_name(),
                                ins=[], outs=[])
                            nop.engine = ins.engine
                            nop.sync_info = mybir.SyncInfo(
                                on_wait=[w], on_update=[])
                            nc.register_instruction(nop)
                            insts.insert(i, nop)
                            i += 1
                        si.on_wait = keep
                i += 1


def prep_inputs(h_i, h_t, mask, W, b, u, S=S, H=H, A=A):
    """Shard + lay out the full inputs for the 8 cores (heavy data fp16)."""
    h_i = np.asarray(h_i, np.float32)
    h_t = np.asarray(h_t, np.float32)
    mask = np.asarray(mask, bool)
    W = np.asarray(W, np.float32)
    b = np.asarray(b, np.float32)
    u = np.asarray(u, np.float32)

    KT = H // 128
    AT = A // 128
    w1t = np.ascontiguousarray(W[:, :H].T).astype(np.float16)   # [H, A]
    cb = h_t @ W[:, H:].T + b                                   # [B, A]
    cb2s = np.ascontiguousarray(
        cb.reshape(B, AT, 128).transpose(2, 1, 0))              # [128, AT, B]
    u2 = np.ascontiguousarray(
        u[:, 0].reshape(AT, 128).T).astype(np.float16)          # [128, AT]

    pos = np.clip(np.cumsum(mask.astype(np.int64), axis=0) - 1, 0, None)
    onehot = (np.arange(B)[None, :, None] == pos[:, None, :]) & mask[:, None, :]
    selall = onehot.astype(np.float16)                          # [B, B, S]
    negall = np.where(mask, np.float16(0), np.float16(NEG))     # [B, S]
    sel33 = np.concatenate([selall, negall[:, None, :]], axis=1)  # [B, B+1, S]

    h16 = h_i.astype(np.float16)
    in_maps = []
    for c in range(NCORES):
        bs = slice(c * BL, (c + 1) * BL)
        # hi5[p, chunk ++ (t, b, s)] = h_i[b, chunk*CI+s, t*128+p]
        hcf = h16[bs].reshape(BL, NCH, CI, KT, 128)
        hi5 = np.ascontiguousarray(
            hcf.transpose(4, 1, 3, 0, 2).reshape(128, KT * BL * S))
        in_maps.append({
            "hi5": hi5,
            "w1t": w1t,
            "cb2": np.ascontiguousarray(cb2s[:, :, bs]),
            "u2": u2,
            "sel": np.ascontiguousarray(sel33[bs].transpose(1, 0, 2)),
        })
    return in_maps


_NC_CACHE = {}


def _get_nc():
    if "nc" not in _NC_CACHE:
        _NC_CACHE["nc"] = build_kernel()
    return _NC_CACHE["nc"]


def kernel(h_i, h_t, mask, W, b, u):
    nc = _get_nc()
    in_maps = prep_inputs(h_i, h_t, mask, W, b, u)
    res = run_bass_kernel_spmd(nc, in_maps, list(range(NCORES)))
    return np.concatenate([res.results[c]["out"] for c in range(NCORES)],
                          axis=0)


# revision 18
# speedup vs baseline: 1.3651x; 1.0413x over previous
"""Trainium2 Bass kernel for nn_Attention_b (tanh-attention with masked_scatter).

Data-parallel over batch: each of 8 NeuronCores owns 4 batches. h_i and W
travel in bf16 (DVE gets its 2x 16-bit path only for bf16; PE bf16 == fp32r
speed), the score pipeline (m, u, y, sel) in fp16 for precision, exp weights
in bf16. End-to-end rel err ~9e-3 vs the 2e-2 gate. The whole per-core h_i
slice stays resident in SBUF; all DMAs are issued up front.

Schedule: phase 1 (score GEMM chunks) runs back-to-back on PE; a warm-up
AllGather brings up the CC rings during the first ~70us; grouped score
AllGathers fire as their chunks complete; each group's masked-scatter +
softmax + weighted-sum (phase 2/3) is interleaved into the instruction
streams at the point where its data is ready, so no engine queue parks
behind an unfinished dependency.
"""
import sys

for _p in ("/opt/trn_rl_repo",):
    if _p not in sys.path:
        sys.path.insert(0, _p)

import numpy as np

import concourse.bacc as bacc
import concourse.tile as tile
from concourse import mybir
from concourse.bass_utils import run_bass_kernel_spmd
from concourse.dve_ops import TENSOR_TENSOR_REDUCE
from concourse.masks import make_identity

NCORES = 8
B, S, H, A = 32, 2048, 1024, 256
BL = B // NCORES          # local batches per core
NEG = np.float32(-60000.0)   # fits fp16; exp(NEG-max) == 0 regardless

f32 = mybir.dt.float32
f16 = mybir.dt.float16
bf16 = mybir.dt.bfloat16

P1LIST = [128, 128, 256, 512, 512, 512]   # phase-1 (GEMM) chunking
CI = 512                                  # phase-2/3 chunking
NCH2 = S // CI
# AllGather groups as column ranges; group g covers phase-2 chunks P2G[g]
AGCOLS = [(0, 512), (512, 1536), (1536, 2048)]
P2G = [[0], [1, 2], [3]]


def build_kernel():
    KT = H // 128             # contraction tiles
    AT = A // 128             # score tiles
    offs = np.concatenate([[0], np.cumsum(P1LIST)]).tolist()
    NCH1 = len(P1LIST)
    assert offs[-1] == S

    nc = bacc.Bacc("TRN2", target_bir_lowering=False, debug=False,
                   num_devices=NCORES)

    hi5 = nc.declare_dram_parameter("hi5", [128, KT * BL * S], bf16,
                                    isOutput=False)
    w1t = nc.declare_dram_parameter("w1t", [H, A], bf16, isOutput=False)
    cb2 = nc.declare_dram_parameter("cb2", [128, AT, BL], f32, isOutput=False)
    u2 = nc.declare_dram_parameter("u2", [128, AT], f16, isOutput=False)
    sel = nc.declare_dram_parameter("sel", [B + 1, BL, S], f16,
                                    isOutput=False)
    out = nc.declare_dram_parameter("out", [BL, H], f32, isOutput=True)

    with tile.TileContext(nc) as tc:
        with (
            tc.tile_pool(name="consts", bufs=1) as cp,
            tc.tile_pool(name="m", bufs=2) as mp,
            tc.tile_pool(name="tiny", bufs=2) as tp,
            tc.tile_pool(name="sely", bufs=2) as syp,
            tc.tile_pool(name="ebc", bufs=5) as ebp,
            tc.tile_pool(name="sacc", bufs=2) as sap,
            tc.tile_pool(name="pz", bufs=2, space="PSUM") as pz,
            tc.tile_pool(name="py", bufs=2, space="PSUM") as py,
            tc.tile_pool(name="dram", bufs=3, space="DRAM") as dp,
        ):
            # ---- preload replicated constants
            w1_sb = cp.tile([128, KT, A], bf16)
            nc.sync.dma_start(
                out=w1_sb, in_=w1t.rearrange("(t p) a -> p t a", p=128))
            u_sb = cp.tile([128, AT], f16)
            nc.sync.dma_start(out=u_sb, in_=u2[:, :])
            cb_sb = cp.tile([128, AT, BL], f32)
            nc.sync.dma_start(out=cb_sb, in_=cb2[:, :, :])
            ident = cp.tile([128, 128], f32)
            make_identity(nc, ident)
            ones_sb = cp.tile([B + 1, 1], f16)
            nc.vector.memset(ones_sb, 1.0)
            wup = cp.tile([128, 512], bf16)
            nc.vector.memset(wup, 0.0)

            # ---- warm-up: bring up CC rings / absorb core skew early
            wg_in = dp.tile([1], f16, tag="wgin")
            nc.scalar.dma_start(out=wg_in.rearrange("(o n) -> o n", o=1),
                                in_=ones_sb[0:1, 0:1])
            wg_out = dp.tile([8], f16, tag="wgout", addr_space="Shared")
            nc.gpsimd.collective_compute(
                "AllGather", mybir.AluOpType.bypass,
                ins=[wg_in[:]], outs=[wg_out[:]],
                replica_groups=[list(range(NCORES))],
            )

            # ---- all input streams issued up front, h_i phase1-chunk-major
            hi_all = cp.tile([128, KT * BL * S], bf16)
            for i in range(NCH1):
                nc.sync.dma_start(
                    out=hi_all[:, KT * BL * offs[i] : KT * BL * offs[i + 1]],
                    in_=hi5[:, KT * BL * offs[i] : KT * BL * offs[i + 1]])
            sel_all = cp.tile([B + 1, BL, S], f16)
            nc.scalar.dma_start(out=sel_all, in_=sel[:, :, :])

            # PE p-state warm-up while DMAs stream
            wup_ps = pz.tile([128, 2, 512], f32, tag="z")
            for i in range(10):
                nc.tensor.matmul(
                    wup_ps[:, i % 2, :], wup[:, 0:128], wup[:, 0:512],
                    start=True, stop=True)

            def hi1(i):   # phase-1 chunk view [128, KT, BL, Ci]
                return hi_all[:, KT * BL * offs[i] : KT * BL * offs[i + 1]] \
                    .rearrange("p (t b s) -> p t b s", t=KT, b=BL)

            def hi2(j, kt, b):   # phase-2/3 slice [128, CI] of (kt, b)
                # column range [j*CI, (j+1)*CI) spans exact phase-1 chunks
                i0 = offs.index(j * CI)
                i1 = offs.index((j + 1) * CI)
                views = []
                for i in range(i0, i1):
                    Ci = P1LIST[i]
                    v = hi1(i)[:, kt, b, :]
                    views.append(v)
                return views   # list of [128, Ci] views covering CI cols

            # ---- per-chunk softmax stats (combined once at the end)
            mall = cp.tile([1, BL, NCH2], f32)
            lall = cp.tile([1, BL, NCH2], f32)
            saccs = [cp.tile([128, KT, BL, 3], f32, name=f"sacc{j}")
                     for j in range(NCH2)]
            y32s = [cp.tile([B + 1, CI], f16, name=f"y32_{j}")
                    for j in range(NCH2)]
            for j in range(NCH2):
                nc.gpsimd.memset(y32s[j][B : B + 1, :], 1.0)

            # per-AllGather-group DRAM score buffers
            ag_ins, ag_outs = [], []
            for g, (c0, c1) in enumerate(AGCOLS):
                wdt = c1 - c0
                agi = dp.tile([BL * wdt], f16, tag=f"agin{g}",
                              name=f"agin{g}")
                ago = dp.tile([B * wdt], f16, tag=f"agout{g}",
                              name=f"agout{g}", addr_space="Shared")
                ag_ins.append(agi)
                ag_outs.append(ago)

            def phase1(i):
                Ci = P1LIST[i]
                hi_sb = hi1(i)
                g = next(gi for gi, (c0, c1) in enumerate(AGCOLS)
                         if c0 <= offs[i] < c1)
                c0, c1 = AGCOLS[g]
                m_r = mp.tile([128, AT, BL, Ci], f16, tag="m")
                for at in range(AT):
                    for bp in range(BL // 2):
                        z_ps = pz.tile([128, 2, Ci], f32, tag="z")
                        for b2 in range(2):
                            b = bp * 2 + b2
                            for kt in range(KT):
                                nc.tensor.matmul(
                                    z_ps[:, b2, :],
                                    w1_sb[:, kt, at * 128 : (at + 1) * 128],
                                    hi_sb[:, kt, b, :],
                                    start=(kt == 0), stop=(kt == KT - 1),
                                )
                        for b2 in range(2):
                            b = bp * 2 + b2
                            nc.scalar.activation(
                                out=m_r[:, at, b, :], in_=z_ps[:, b2, :],
                                func=mybir.ActivationFunctionType.Tanh,
                                bias=cb_sb[:, at, b : b + 1], scale=1.0,
                            )
                y_sb = tp.tile([1, BL, Ci], f16, tag="ysb")
                for bp in range(BL // 2):
                    y_ps = py.tile([1, 2, Ci], f32, tag="y")
                    for b2 in range(2):
                        b = bp * 2 + b2
                        for at in range(AT):
                            nc.tensor.matmul(
                                y_ps[:, b2 : b2 + 1, :],
                                u_sb[:, at : at + 1],
                                m_r[:, at, b : b + 1, :],
                                start=(at == 0), stop=(at == AT - 1),
                            )
                    nc.scalar.activation(
                        out=y_sb[:, 2 * bp : 2 * bp + 2, :], in_=y_ps,
                        func=mybir.ActivationFunctionType.Copy)
                # stage this chunk's scores into its group's DRAM buffer
                loc = offs[i] - c0
                nc.scalar.dma_start(
                    out=ag_ins[g].rearrange("(o b s) -> o b s", o=1,
                                            s=c1 - c0)
                                  [:, :, loc : loc + Ci],
                    in_=y_sb)

            def ag_fire(g):
                c0, c1 = AGCOLS[g]
                nc.gpsimd.collective_compute(
                    "AllGather", mybir.AluOpType.bypass,
                    ins=[ag_ins[g][:]], outs=[ag_outs[g][:]],
                    replica_groups=[list(range(NCORES))],
                )
                # land gathered scores via the idle sync queue
                for j in P2G[g]:
                    loc = j * CI - c0
                    nc.sync.dma_start(
                        out=y32s[j][:B, :],
                        in_=ag_outs[g].rearrange("(q s) -> q s", s=c1 - c0)
                                      [:, loc : loc + CI])

            def phase23(j):
                off = j * CI
                sel_c = sel_all[:, :, off : off + CI]
                y32 = y32s[j]
                selY = syp.tile([B + 1, BL, CI], f16, tag="selY")
                nc.gpsimd.tensor_mul(
                    selY, sel_c,
                    y32.rearrange("q (o s) -> q o s", o=1)
                       .broadcast_to([B + 1, BL, CI]))
                cmax = tp.tile([1, BL], f32, tag="cmax")
                bt_hs = []
                for hf in range(2):
                    bt_ps = py.tile([1, 2, CI], f32, tag="y")
                    for b2 in range(2):
                        nc.tensor.matmul(
                            bt_ps[:, b2 : b2 + 1, :], ones_sb,
                            selY[:, 2 * hf + b2 : 2 * hf + b2 + 1, :],
                            start=True, stop=True)
                    nc.vector.tensor_reduce(
                        out=cmax.rearrange("p (b o) -> p b o", o=1)
                                [:, 2 * hf : 2 * hf + 2],
                        in_=bt_ps,
                        axis=mybir.AxisListType.X, op=mybir.AluOpType.max)
                    bt_hs.append(bt_ps)
                nc.vector.tensor_copy(mall[:, :, j], cmax)
                nmnew = tp.tile([1, BL], f32, tag="nmnew")
                nc.vector.tensor_scalar_mul(nmnew, cmax, -1.0)
                sacc_j = saccs[j]
                ttr_scr = tp.tile([128, 1], bf16, tag="ttrscr")
                for b in range(BL):
                    e4 = tp.tile([1, CI], bf16, tag="e4", bufs=5)
                    nc.scalar.activation(
                        out=e4, in_=bt_hs[b // 2][:, b % 2, :],
                        func=mybir.ActivationFunctionType.Exp,
                        bias=nmnew[:, b : b + 1], scale=1.0,
                        accum_out=lall[:, b, j : j + 1])
                    e_bc = ebp.tile([128, CI], bf16, tag="ebc")
                    nc.gpsimd.partition_broadcast(e_bc, e4)
                    # weighted sum on DVE, split across the phase-1 views
                    for kt in range(KT):
                        views = hi2(j, kt, b)
                        cw = 0
                        for vi, v in enumerate(views):
                            wv = v.shape[-1]
                            nc.vector._custom_dve(
                                TENSOR_TENSOR_REDUCE,
                                out=ttr_scr.broadcast_to([128, wv]),
                                in0=v,
                                in1=e_bc[:, cw : cw + wv],
                                s0=0.0, s1=1.0,
                                accum_out=sacc_j[:, kt, b, vi : vi + 1],
                            )
                            cw += wv

            # ---- schedule: phase1 back-to-back with phase2/3 interleaved
            # at the points where their collectives have completed
            phase1(0); phase1(1); phase1(2)
            ag_fire(0)                      # cols 0-512 (rides ring ~t70)
            phase1(3); phase1(4)
            phase23(0)                      # interleaves before phase1(5)
            ag_fire(1)                      # cols 512-1536
            phase1(5)
            phase23(1); phase23(2)
            ag_fire(2)                      # cols 1536-2048
            phase23(3)

            # ---- finalize: combine chunk partials, divide, transpose, store
            M = tp.tile([1, BL], f32, tag="cmax")
            nc.vector.tensor_reduce(
                out=M.rearrange("p (b o) -> p b o", o=1), in_=mall,
                axis=mybir.AxisListType.X, op=mybir.AluOpType.max)
            nM = tp.tile([1, BL], f32, tag="nmnew")
            nc.vector.tensor_scalar_mul(nM, M, -1.0)
            w = tp.tile([1, BL, NCH2], f32, tag="w")
            for b in range(BL):
                nc.scalar.activation(
                    out=w[:, b, :], in_=mall[:, b, :],
                    func=mybir.ActivationFunctionType.Exp,
                    bias=nM[:, b : b + 1], scale=1.0)
            wl = tp.tile([1, BL, NCH2], f32, tag="wl")
            nc.vector.tensor_mul(wl, w, lall)
            lsum = tp.tile([1, BL], f32, tag="lsum")
            nc.vector.tensor_reduce(
                out=lsum.rearrange("p (b o) -> p b o", o=1), in_=wl,
                axis=mybir.AxisListType.X, op=mybir.AluOpType.add)
            il = tp.tile([1, BL], f32, tag="il")
            nc.vector.reciprocal(il, lsum)
            wn = tp.tile([1, BL, NCH2], f32, tag="wn")
            for b in range(BL):
                nc.vector.tensor_scalar_mul(wn[:, b, :], w[:, b, :],
                                            il[:, b : b + 1])
            wbc = ebp.tile([128, BL * NCH2], f32, tag="wbcf")
            nc.gpsimd.partition_broadcast(
                wbc, wn.rearrange("p b n -> p (b n)"))
            wbc_v = wbc.rearrange("p (b n) -> p b n", b=BL)
            sfin = sap.tile([128, KT, BL], f32, tag="sacc")
            nviews = [offs.index((j + 1) * CI) - offs.index(j * CI)
                      for j in range(NCH2)]
            for j in range(NCH2):
                for b in range(BL):
                    # fold the per-view sub-accumulators as we combine
                    nparts = nviews[j]
                    for vi in range(nparts):
                        tmp = tp.tile([128, KT], f32, tag="ftmp")
                        nc.vector.tensor_scalar_mul(
                            tmp, saccs[j][:, :, b, vi],
                            wbc_v[:, b, j : j + 1])
                        if j == 0 and vi == 0:
                            nc.vector.tensor_copy(sfin[:, :, b], tmp)
                        else:
                            nc.vector.tensor_add(
                                sfin[:, :, b], sfin[:, :, b], tmp)
            t_ps = py.tile([KT * BL, 128], f32, tag="y")
            nc.tensor.transpose(
                t_ps, sfin.rearrange("p t b -> p (t b)"), ident)
            t_sb = tp.tile([KT * BL, 128], f32, tag="tsb")
            nc.vector.tensor_copy(t_sb, t_ps)
            for t in range(KT):
                nc.sync.dma_start(
                    out=out[:, t * 128 : (t + 1) * 128],
                    in_=t_sb[t * BL : (t + 1) * BL, :])

    nc.compile()
    _split_pe_waits(nc)
    return nc


def _split_pe_waits(nc):
    """TRN2 PE instructions (S3_LW encoding) take a single sync-wait slot.
    Bacc's legalization misses some Matmults; hoist excess waits onto
    dedicated PE NoOps inserted directly before the offender."""
    for f in nc.m.functions:
        for bb in f.blocks:
            insts = bb.instructions
            i = 0
            while i < len(insts):
                ins = insts[i]
                if type(ins).__name__ in ("InstMatmult", "InstNoOp") and \
                        ins.engine == mybir.EngineType.PE:
                    si = ins.sync_info
                    if si is not None and len(si.on_wait) > 1:
                        extra, keep = si.on_wait[:-1], si.on_wait[-1:]
                        for w in extra:
                            nop = mybir.InstNoOp(
                                name=nc.get_next_instruction_name(),
                                ins=[], outs=[])
                            nop.engine = ins.engine
                            nop.sync_info = mybir.SyncInfo(
                                on_wait=[w], on_update=[])
                            nc.register_instruction(nop)
                            insts.insert(i, nop)
                            i += 1
                        si.on_wait = keep
                i += 1


def prep_inputs(h_i, h_t, mask, W, b, u):
    """Shard + lay out the full inputs for the 8 cores."""
    import ml_dtypes
    h_i = np.asarray(h_i, np.float32)
    h_t = np.asarray(h_t, np.float32)
    mask = np.asarray(mask, bool)
    W = np.asarray(W, np.float32)
    b = np.asarray(b, np.float32)
    u = np.asarray(u, np.float32)

    KT = H // 128
    AT = A // 128
    offs = np.concatenate([[0], np.cumsum(P1LIST)]).astype(int)
    w1t = np.ascontiguousarray(W[:, :H].T).astype(ml_dtypes.bfloat16)
    cb = h_t @ W[:, H:].T + b                                   # [B, A]
    cb2s = np.ascontiguousarray(
        cb.reshape(B, AT, 128).transpose(2, 1, 0))              # [128, AT, B]
    u2 = np.ascontiguousarray(
        u[:, 0].reshape(AT, 128).T).astype(np.float16)          # [128, AT]

    pos = np.clip(np.cumsum(mask.astype(np.int64), axis=0) - 1, 0, None)
    onehot = (np.arange(B)[None, :, None] == pos[:, None, :]) & mask[:, None, :]
    selall = onehot.astype(np.float16)                          # [B, B, S]
    negall = np.where(mask, np.float16(0), np.float16(NEG))     # [B, S]
    sel33 = np.concatenate([selall, negall[:, None, :]], axis=1)  # [B, B+1, S]

    h16 = h_i.astype(ml_dtypes.bfloat16)
    in_maps = []
    for c in range(NCORES):
        bs = slice(c * BL, (c + 1) * BL)
        # hi5[p, chunk ++ (t, b, s)] = h_i[b, off_i+s, t*128+p]
        hcf = h16[bs].reshape(BL, S, KT, 128)
        blocks = []
        for ci, off in zip(P1LIST, offs[:-1]):
            hc = hcf[:, off : off + ci]                     # [BL, ci, KT, 128]
            blocks.append(hc.transpose(3, 2, 0, 1).reshape(128, KT * BL * ci))
        hi5 = np.ascontiguousarray(np.concatenate(blocks, axis=1))
        in_maps.append({
            "hi5": hi5,
            "w1t": w1t,
            "cb2": np.ascontiguousarray(cb2s[:, :, bs]),
            "u2": u2,
            "sel": np.ascontiguousarray(sel33[bs].transpose(1, 0, 2)),
        })
    return in_maps


_NC_CACHE = {}


def _get_nc():
    if "nc" not in _NC_CACHE:
        _NC_CACHE["nc"] = build_kernel()
    return _NC_CACHE["nc"]


def kernel(h_i, h_t, mask, W, b, u):
    nc = _get_nc()
    in_maps = prep_inputs(h_i, h_t, mask, W, b, u)
    res = run_bass_kernel_spmd(nc, in_maps, list(range(NCORES)))
    return np.concatenate([res.results[c]["out"] for c in range(NCORES)],
                          axis=0)
